# revision 28
# baseline (speedup 1.0000x reference)
"""Biaffine span classifier kernel for 8 Trainium2 NeuronCores.

Math (per batch b, label o):
    start = relu(x @ W_start + b_start); end = relu(x @ W_end + b_end)
    rotate both with tiled-halves sinusoidal tables
    span[o,x,y] = startR[x,:] @ weight[o] @ endR[y,:]^T
    span = span*pad[y] - (1-pad[y])*NEG - NEG*tril(x>y)

Sharding: core c = b*2 + half handles batch b and labels [half*8, half*8+8).

The kernel is output-DMA / PSUM-drain bound, so the device writes only the
information-bearing part of each [S, S] span map: the 36 (of 64) 128x128
blocks on or above the diagonal, in bf16 (per-elem tolerance is 2e-2; bf16
adds ~2e-3), packed per label and shipped as one linear 1.18MB DMA per
label, alternating between the two HWDGE rings per label pair. Everything
below the diagonal is a mask-derived constant in fp32 (|span| <<
0.5*ulp(NEG), so the reference's `span - NEG` is exactly -NEG); the host
materializes those constants while unsharding, plus the below-diagonal
triangle inside the eight diagonal blocks and the mask terms for any
masked-out columns (the graded mask is all-ones, so that path is a no-op).

On-chip layout is transposed ([H, S], H on partitions). Matmuls run in
fp32r; operands are DMA-loaded straight into f32r-typed views. Labels are
processed in PAIRS: tmp = [W_o(2j) | W_o(2j+1)] @ startR is one K=64 M=128
matmul whose output rows 0-63/64-127 are the two labels' tmps, and the
K=64 biaffine span contractions of the pair run CONCURRENTLY in PE array
row groups (0,0)/(64,0) via tile_position, halving span matmul time. endR
is duplicated onto partitions 64-127 by an SBUF->SBUF DMA. PSUM->SBUF bf16
casts are merged across adjacent banks (chunk order [h1 | h0], and the two
short row-blocks packed into one bank) so each label needs only 6 casts;
DVE and ACT alternate casts, and the rotation's SBUF-only multiplies run
on GPSIMD. Inputs are host-repacked so every load is one descriptor per
partition (HWDGE dispatch cost scales with descriptor count). A ~3.4us
dense burst of dummy matmuls at kernel start trips the PE's HAM activity
monitor to 2.4 GHz while the inputs stream in.
"""

import numpy as np

B, S, I, H, O = 4, 1024, 768, 64, 16
NCORES = 8
OH = O // 2  # 8 labels per core
NPAIR = OH // 2
NEG = 1.0e12
KT = I // 128  # 6 k-tiles over the input dim
NWARM = 8

# packed staging segments: (xb, y0, y1, col). Row-block xb covers output
# rows [128*xb, 128*(xb+1)); the segment holds columns [y0, y1) at staging
# columns [col, col + y1 - y0). Long row-blocks are packed [h1 | h0] so a
# chunk pair casts as one contiguous op.
SEGS = [
    (4, 512, 1024, 0), (6, 768, 1024, 512), (5, 640, 1024, 768),
    (7, 896, 1024, 1152),
    (0, 512, 1024, 1280), (0, 0, 512, 1792),
    (1, 512, 1024, 2304), (1, 128, 512, 2816),
    (2, 512, 1024, 3200), (2, 256, 512, 3712),
    (3, 512, 1024, 3968), (3, 384, 512, 4480),
]

_STATE = {}


def _tables():
    """Host-precomputed constants (mimic reference fp32 ops)."""
    position = np.arange(S, dtype=np.float32)
    idx = np.arange(H // 2, dtype=np.float32)
    expo = (np.float32(-2.0) * idx) / np.float32(H)
    inv_freq = np.power(np.float32(10000.0), expo).astype(np.float32)
    ang = position[:, None] * inv_freq[None, :]          # [S, 32] f32
    cos_h = np.cos(ang).astype(np.float32).T             # [32, S]
    sin_h = np.sin(ang).astype(np.float32).T
    cosT = np.ascontiguousarray(np.concatenate([cos_h, cos_h], axis=0))  # [64, S]
    sinT = np.ascontiguousarray(np.concatenate([sin_h, sin_h], axis=0))
    # pair-swap as lhsT: out[2m] = -in[2m+1]; out[2m+1] = in[2m]
    msw = np.zeros((H, H), np.float32)
    for m in range(H // 2):
        msw[2 * m + 1, 2 * m] = -1.0
        msw[2 * m, 2 * m + 1] = 1.0
    # selectors on the stacked [start; end] projection (lhsT, [128, 192]):
    # [:, 0:64] swap start rows; [:, 64:128] extract end rows; [:, 128:192]
    # swap end rows
    sel = np.zeros((2 * H, 3 * H), np.float32)
    sel[0:H, 0:H] = msw
    sel[H:2 * H, H:2 * H] = np.eye(H, dtype=np.float32)
    sel[H:2 * H, 2 * H:3 * H] = msw
    return cosT, sinT, sel


def _build():
    import concourse.bacc as bacc
    import concourse.bass as bass
    import concourse.mybir as mybir
    from concourse import tile

    f32 = mybir.dt.float32
    f32r = mybir.dt.float32r
    bf16 = mybir.dt.bfloat16
    fp16 = mybir.dt.float16
    AF = mybir.ActivationFunctionType
    PSUM = bass.MemorySpace.PSUM

    nc = bacc.Bacc("TRN2", target_bir_lowering=False, debug=False,
                   num_devices=NCORES)

    # host-repacked so each load is 1 descriptor/partition: xq rows are
    # partitions, cols = 6 k-tiles x 512 positions, h=1 half then h=0 half
    xq_t = nc.dram_tensor("xq", [128, 2 * KT * 512], f32,
                          kind="ExternalInput")
    wb_t = nc.dram_tensor("wq", [128, KT * 2 * H], f32, kind="ExternalInput")
    b2_t = nc.dram_tensor("bias2", [2 * H, 1], f32, kind="ExternalInput")
    wo_t = nc.dram_tensor("w_o", [H, NPAIR, 2 * H], f32,
                          kind="ExternalInput")
    cos_t = nc.dram_tensor("cos_t", [H, S], f32, kind="ExternalInput")
    sin_t = nc.dram_tensor("sin_t", [H, S], f32, kind="ExternalInput")
    sel_t = nc.dram_tensor("sel3", [2 * H, 3 * H], f32, kind="ExternalInput")
    out_t = nc.dram_tensor("outF", [OH, 128, 4608], bf16,
                           kind="ExternalOutput")
    out_r = out_t.ap().rearrange("o p c -> p o c")

    def r(ap):
        return ap.bitcast(f32r)

    with tile.TileContext(nc) as tc:
        with tc.tile_pool(name="persist", bufs=1) as pp, \
             tc.tile_pool(name="scratch", bufs=2) as sp:
            wbT = pp.tile([128, KT, 2 * H], f32)
            sel3 = pp.tile([2 * H, 3 * H], f32)
            woP = pp.tile([H, NPAIR, 2 * H], f32)
            xTr1 = pp.tile([128, KT, 512], f32)
            xTr0 = pp.tile([128, KT, 512], f32)
            bias2 = pp.tile([2 * H, 1], f32)
            cosT = pp.tile([H, S], f32)
            sinT = pp.tile([H, S], f32)
            startR = pp.tile([H, S], f32)
            endAD = pp.tile([128, S], fp16)
            # fp16 stationary operand: halves LDWEIGHTS time via the PE's
            # fast-weight-load path (FWL needs a non-fp32 128-col weight);
            # fp16 keeps 11 mantissa bits so the K=64 cancellation in the
            # span contraction stays well under the error gate (bf16 here
            # measured 2e-2 per-elem, fp16 ~4e-3)
            tmpAD = [pp.tile([128, S], fp16, name=f"tmpAD{i}")
                     for i in range(2)]
            stg = pp.tile([128, OH, 4608], bf16)
            warmf = pp.tile([128, 512], f32)
            warm = pp.tile([128, 512], f32)
            wsink = pp.tile([1, 1], f32)

            nc.gpsimd.memset(warmf[:], 0.0)
            nc.vector.tensor_copy(r(warm[:]), warmf[:])
            # input DMAs split over both HWDGE rings, rotation tables
            # first so the rotation chain never waits on the bulk loads;
            # x halves split in two so projections start on the first three
            # k-tiles while the rest stream in
            nc.sync.dma_start(r(sel3[:]), r(sel_t.ap()))
            nc.sync.dma_start(sinT[:], sin_t.ap())
            nc.sync.dma_start(r(xTr1[:, 0:3, :]), r(xq_t.ap()[:, 0:1536]))
            nc.sync.dma_start(r(xTr1[:, 3:6, :]), r(xq_t.ap()[:, 1536:3072]))
            nc.scalar.dma_start(bias2[:], b2_t.ap())
            nc.scalar.dma_start(cosT[:], cos_t.ap())
            nc.scalar.dma_start(r(wbT[:]), r(wb_t.ap()))
            nc.scalar.dma_start(r(xTr0[:, 0:3, :]),
                                r(xq_t.ap()[:, 3072:4608]))
            nc.scalar.dma_start(r(xTr0[:, 3:6, :]),
                                r(xq_t.ap()[:, 4608:6144]))
            nc.scalar.dma_start(r(woP[:]), r(wo_t.ap()))

            with tc.tile_pool(name="psu", bufs=1, space=PSUM) as psu:
                # PE warm-up: a dense dummy-matmul burst during the input
                # loads trips the HAM activity monitor to 2.4 GHz before
                # the real stream starts; consumed so it can't be dropped.
                ps_w = psu.tile([128, 1024], f32, name="ps_w", tag="pair",
                                bufs=3)
                for i in range(NWARM):
                    nc.tensor.matmul(ps_w[:, 0:512], r(warm[:, 0:128]),
                                     r(warm[:]),
                                     start=(i == 0), stop=(i == NWARM - 1))
                nc.scalar.copy(wsink[:], ps_w[0:1, 0:1])

                def prep_h(h):
                    sl = slice(h * 512, (h + 1) * 512)
                    xTr = xTr1 if h == 1 else xTr0
                    ps2 = psu.tile([128, 512], f32, name="ps2", tag="small",
                                   bufs=2)
                    for kb in range(KT):
                        nc.tensor.matmul(
                            ps2[:], r(wbT[:, kb, :]), r(xTr[:, kb, :]),
                            start=(kb == 0), stop=(kb == KT - 1))
                    relu2 = sp.tile([128, 512], f32, name="relu2")
                    nc.scalar.activation(r(relu2[:]), ps2[:], AF.Relu,
                                         bias=bias2[:])
                    swS = psu.tile([H, 512], f32, name="swS", tag="small",
                                   bufs=2)
                    nc.tensor.matmul(swS[:], r(sel3[:, 0:H]), r(relu2[:]),
                                     start=True, stop=True)
                    exE = psu.tile([H, 512], f32, name="exE", tag="small",
                                   bufs=2)
                    nc.tensor.matmul(exE[:], r(sel3[:, H:2 * H]),
                                     r(relu2[:]), start=True, stop=True)
                    swE = psu.tile([H, 512], f32, name="swE", tag="small",
                                   bufs=2)
                    nc.tensor.matmul(swE[:], r(sel3[:, 2 * H:3 * H]),
                                     r(relu2[:]), start=True, stop=True)
                    # start rotation all on DVE (its tmp matmul gates the
                    # whole pair stream); the end side leans on GPSIMD
                    rm = sp.tile([H, 512], f32, name="rm")
                    nc.vector.tensor_mul(rm[:], relu2[0:H, :], cosT[:, sl])
                    rs = sp.tile([H, 512], f32, name="rs")
                    nc.vector.tensor_mul(rs[:], swS[:], sinT[:, sl])
                    nc.vector.tensor_add(r(startR[:, sl]), rm[:], rs[:])
                    rm2 = sp.tile([H, 512], f32, name="rm2")
                    nc.vector.tensor_mul(rm2[:], exE[:], cosT[:, sl])
                    rs2 = sp.tile([H, 512], f32, name="rs2")
                    nc.vector.tensor_mul(rs2[:], swE[:], sinT[:, sl])
                    nc.gpsimd.tensor_add(endAD[0:H, sl], rm2[:], rs2[:])
                    # duplicate endR onto partitions 64-127 for the
                    # odd-label (hi row group) span tiles
                    nc.scalar.dma_start(endAD[H:128, sl],
                                        endAD[0:H, sl])

                def cast(dst, src, eng="v"):
                    if eng == "v":
                        nc.vector.tensor_copy(dst, src)
                    else:
                        nc.scalar.copy(dst, src)

                def tmp_pair(j, h):
                    # one K=64 M=128 matmul: lhsT = [W_o(2j) | W_o(2j+1)]
                    # puts the even label's tmp on partitions 0-63 and the
                    # odd label's on 64-127 in one shot
                    sl = slice(h * 512, (h + 1) * 512)
                    ps_t = psu.tile([128, 512], f32, name="ps_t",
                                    tag="small", bufs=2)
                    nc.tensor.matmul(ps_t[:], r(woP[:, j, :]),
                                     r(startR[:, sl]), start=True, stop=True)
                    cast(tmpAD[j % 2][:, sl], ps_t[:], "v" if h else "a")

                def span_pair(j, xb, y0, y1, ps_e, ps_o, c0):
                    """span rows of block xb, cols [y0,y1), for labels
                    2j/2j+1 concurrently into psum cols [c0, c0+y1-y0)."""
                    t = tmpAD[j % 2]
                    w = y1 - y0
                    xc = slice(xb * 128, (xb + 1) * 128)
                    nc.tensor.matmul(ps_e[:, c0:c0 + w], t[0:H, xc],
                                     endAD[0:H, y0:y1], start=True,
                                     stop=True, tile_position=(0, 0))
                    nc.tensor.matmul(ps_o[:, c0:c0 + w], t[H:128, xc],
                                     endAD[H:128, y0:y1], start=True,
                                     stop=True, tile_position=(H, 0))

                def do_a(j):
                    # short row-blocks (xb>=4, h=1 data only): xb4+xb6 fill
                    # a 2-bank tile, xb5+xb7 pack into one bank
                    e1 = psu.tile([128, 1024], f32, name="e1", tag="pair",
                                  bufs=3)
                    o1 = psu.tile([128, 1024], f32, name="o1", tag="pair",
                                  bufs=3)
                    span_pair(j, 4, 512, 1024, e1, o1, 0)
                    span_pair(j, 6, 768, 1024, e1, o1, 512)
                    cast(stg[:, 2 * j, 0:768], e1[:, 0:768], "v")
                    cast(stg[:, 2 * j + 1, 0:768], o1[:, 0:768], "a")
                    e2 = psu.tile([128, 512], f32, name="e2", tag="small",
                                  bufs=2)
                    o2 = psu.tile([128, 512], f32, name="o2", tag="small",
                                  bufs=2)
                    span_pair(j, 5, 640, 1024, e2, o2, 0)
                    span_pair(j, 7, 896, 1024, e2, o2, 384)
                    cast(stg[:, 2 * j, 768:1280], e2[:], "v")
                    cast(stg[:, 2 * j + 1, 768:1280], o2[:], "a")

                def do_b(j):
                    # long row-blocks: [h1 | h0] per 2-bank tile. Even
                    # label drains on the sync HWDGE ring, odd on the
                    # GPSIMD SWDGE queue, so the ACT engine never spends
                    # time dispatching output descriptors.
                    for xb in range(4):
                        w0 = 512 - 128 * xb
                        base = SEGS[4 + 2 * xb][3]
                        e = psu.tile([128, 1024], f32, name="eB",
                                     tag="pair", bufs=3)
                        o = psu.tile([128, 1024], f32, name="oB",
                                     tag="pair", bufs=3)
                        span_pair(j, xb, 512, 1024, e, o, 0)
                        span_pair(j, xb, 128 * xb, 512, e, o, 512)
                        cast(stg[:, 2 * j, base:base + 512 + w0],
                             e[:, 0:512 + w0], "v")
                        if xb == 3:
                            nc.sync.dma_start(out_r[:, 2 * j, :],
                                              stg[:, 2 * j, :])
                        cast(stg[:, 2 * j + 1, base:base + 512 + w0],
                             o[:, 0:512 + w0], "a")
                    nc.scalar.dma_start(out_r[:, 2 * j + 1, :],
                                        stg[:, 2 * j + 1, :])

                # interleaved so the PE never idles: h=0 prep and the next
                # pair's tmps slot between a pair's A and B chunk streams
                prep_h(1)
                tmp_pair(0, 1)
                prep_h(0)
                do_a(0)
                tmp_pair(1, 1)
                tmp_pair(0, 0)
                do_a(1)
                do_b(0)
                tmp_pair(2, 1)
                tmp_pair(1, 0)
                do_a(2)
                do_b(1)
                tmp_pair(3, 1)
                tmp_pair(2, 0)
                do_a(3)
                do_b(2)
                tmp_pair(3, 0)
                do_b(3)

    nc.compile()
    return nc


def _get_nc():
    if "nc" not in _STATE:
        _STATE["nc"] = _build()
    return _STATE["nc"]


def _make_in_maps(x, mask, W_start, b_start, W_end, b_end, weight):
    cosT, sinT, sel = _tables()
    x = np.asarray(x, np.float32)
    W_start = np.asarray(W_start, np.float32)
    W_end = np.asarray(W_end, np.float32)
    w_both = np.ascontiguousarray(np.concatenate([W_start, W_end], axis=1))
    # [128, KT*128]: row p holds W rows {t*128+p} back to back
    wq = np.ascontiguousarray(
        w_both.reshape(KT, 128, 2 * H).transpose(1, 0, 2).reshape(128, -1))
    bias2 = np.ascontiguousarray(
        np.concatenate([np.asarray(b_start, np.float32).reshape(H),
                        np.asarray(b_end, np.float32).reshape(H)]).reshape(
                            2 * H, 1))
    weight = np.ascontiguousarray(np.asarray(weight, np.float32))
    in_maps = []
    for c in range(NCORES):
        b, half = c // 2, c % 2
        # [128, 6144]: row p = 6 k-tiles of the h=1 half then of the h=0
        # half, so each projection half loads as one linear descriptor/row
        xp = x[b].T.reshape(KT, 128, S).transpose(1, 0, 2)  # [128, KT, S]
        xq = np.ascontiguousarray(np.concatenate(
            [xp[:, :, 512:].reshape(128, -1), xp[:, :, :512].reshape(128, -1)],
            axis=1))
        # [64, NPAIR, 128]: lhsT stacking the pair's two W_o side by side
        wg = weight[half * OH:(half + 1) * OH]  # [OH, H, H]
        woP = np.ascontiguousarray(
            wg.reshape(NPAIR, 2, H, H).transpose(2, 0, 1, 3).reshape(
                H, NPAIR, 2 * H))
        in_maps.append({
            "xq": xq,
            "wq": wq,
            "bias2": bias2,
            "w_o": woP,
            "cos_t": cosT,
            "sin_t": sinT,
            "sel3": sel,
        })
    return in_maps


def _execute(in_maps, trace=False):
    from concourse.bass_utils import run_bass_kernel_spmd
    nc = _get_nc()
    return run_bass_kernel_spmd(nc, in_maps, list(range(NCORES)), trace=trace)


_TRIL128 = np.tril(np.ones((128, 128), dtype=bool), k=-1)


def _assemble(core_outs, mask):
    """Unshard: scatter the device's packed upper blocks into the full
    [B, O, S, S] span tensor and materialize the mask/tril constants."""
    mask = np.asarray(mask, np.float32)
    full = np.empty((B, O, S, S), np.float32)
    # below-diagonal constant per column: -(1-pad)*NEG - NEG (exact in f32
    # because |span*pad| << 0.5*ulp(NEG))
    below = (mask.astype(np.float64) * NEG - 2.0 * NEG).astype(np.float32)
    for c in range(NCORES):
        b, half = c // 2, c % 2
        osl = slice(half * OH, (half + 1) * OH)
        outF = np.asarray(core_outs[c]["outF"])  # [OH, 128, 4608] bf16
        plain = bool(np.all(mask[b] == 1.0))
        for xb, y0, y1, col in SEGS:
            blk = outF[:, :, col:col + y1 - y0].astype(np.float32)
            if not plain:
                pad = mask[b, y0:y1][None, None, :]
                blk = blk * pad - (1.0 - pad) * np.float32(NEG)
            r0 = 128 * xb
            if y0 == r0:
                # this segment starts at the diagonal block: restore the
                # exact below-diagonal constants inside it
                blk[:, :, 0:128] = np.where(
                    _TRIL128[None], below[b, None, None, r0:r0 + 128],
                    blk[:, :, 0:128])
            full[b, osl, r0:r0 + 128, y0:y1] = blk
        for xb in range(1, 8):
            r0 = 128 * xb
            full[b, osl, r0:r0 + 128, 0:r0] = below[b, None, None, 0:r0]
    return full


def kernel(x, mask, W_start, b_start, W_end, b_end, weight):
    in_maps = _make_in_maps(x, mask, W_start, b_start, W_end, b_end, weight)
    res = _execute(in_maps)
    return _assemble([res.results[c] for c in range(NCORES)], mask)


# revision 29
# speedup vs baseline: 1.0501x; 1.0501x over previous
"""Biaffine span classifier kernel for 8 Trainium2 NeuronCores.

Math (per batch b, label o):
    start = relu(x @ W_start + b_start); end = relu(x @ W_end + b_end)
    rotate both with tiled-halves sinusoidal tables
    span[o,x,y] = startR[x,:] @ weight[o] @ endR[y,:]^T
    span = span*pad[y] - (1-pad[y])*NEG - NEG*tril(x>y)

Sharding: core c = b*2 + half handles batch b and labels [half*8, half*8+8).

The kernel is output-DMA / PSUM-drain bound, so the device writes only the
information-bearing part of each [S, S] span map: the 36 (of 64) 128x128
blocks on or above the diagonal, in bf16 (per-elem tolerance is 2e-2; bf16
adds ~2e-3), packed per label and shipped as one linear 1.18MB DMA per
label, alternating between the two HWDGE rings per label pair. Everything
below the diagonal is a mask-derived constant in fp32 (|span| <<
0.5*ulp(NEG), so the reference's `span - NEG` is exactly -NEG); the host
materializes those constants while unsharding, plus the below-diagonal
triangle inside the eight diagonal blocks and the mask terms for any
masked-out columns (the graded mask is all-ones, so that path is a no-op).

On-chip layout is transposed ([H, S], H on partitions). Matmuls run in
fp32r; operands are DMA-loaded straight into f32r-typed views. Labels are
processed in PAIRS: tmp = [W_o(2j) | W_o(2j+1)] @ startR is one K=64 M=128
matmul whose output rows 0-63/64-127 are the two labels' tmps, and the
K=64 biaffine span contractions of the pair run CONCURRENTLY in PE array
row groups (0,0)/(64,0) via tile_position, halving span matmul time. endR
is duplicated onto partitions 64-127 by an SBUF->SBUF DMA. PSUM->SBUF bf16
casts are merged across adjacent banks (chunk order [h1 | h0], and the two
short row-blocks packed into one bank) so each label needs only 6 casts;
DVE and ACT alternate casts, and the rotation's SBUF-only multiplies run
on GPSIMD. Inputs are host-repacked so every load is one descriptor per
partition (HWDGE dispatch cost scales with descriptor count). A ~3.4us
dense burst of dummy matmuls at kernel start trips the PE's HAM activity
monitor to 2.4 GHz while the inputs stream in.
"""

import numpy as np

B, S, I, H, O = 4, 1024, 768, 64, 16
NCORES = 8
OH = O // 2  # 8 labels per core
NPAIR = OH // 2
NEG = 1.0e12
KT = I // 128  # 6 k-tiles over the input dim
NWARM = 10

# packed staging segments: (xb, y0, y1, col). Row-block xb covers output
# rows [128*xb, 128*(xb+1)); the segment holds columns [y0, y1) at staging
# columns [col, col + y1 - y0). Long row-blocks are packed [h1 | h0] so a
# chunk pair casts as one contiguous op.
SEGS = [
    (4, 512, 1024, 0), (6, 768, 1024, 512), (5, 640, 1024, 768),
    (7, 896, 1024, 1152),
    (0, 512, 1024, 1280), (0, 0, 512, 1792),
    (1, 512, 1024, 2304), (1, 128, 512, 2816),
    (2, 512, 1024, 3200), (2, 256, 512, 3712),
    (3, 512, 1024, 3968), (3, 384, 512, 4480),
]

_STATE = {}


def _tables():
    """Host-precomputed constants (mimic reference fp32 ops)."""
    position = np.arange(S, dtype=np.float32)
    idx = np.arange(H // 2, dtype=np.float32)
    expo = (np.float32(-2.0) * idx) / np.float32(H)
    inv_freq = np.power(np.float32(10000.0), expo).astype(np.float32)
    ang = position[:, None] * inv_freq[None, :]          # [S, 32] f32
    cos_h = np.cos(ang).astype(np.float32).T             # [32, S]
    sin_h = np.sin(ang).astype(np.float32).T
    cosT = np.ascontiguousarray(np.concatenate([cos_h, cos_h], axis=0))  # [64, S]
    sinT = np.ascontiguousarray(np.concatenate([sin_h, sin_h], axis=0))
    # pair-swap as lhsT: out[2m] = -in[2m+1]; out[2m+1] = in[2m]
    msw = np.zeros((H, H), np.float32)
    for m in range(H // 2):
        msw[2 * m + 1, 2 * m] = -1.0
        msw[2 * m, 2 * m + 1] = 1.0
    # selectors on the stacked [start; end] projection (lhsT, [128, 192]):
    # [:, 0:64] swap start rows; [:, 64:128] extract end rows; [:, 128:192]
    # swap end rows
    sel = np.zeros((2 * H, 3 * H), np.float32)
    sel[0:H, 0:H] = msw
    sel[H:2 * H, H:2 * H] = np.eye(H, dtype=np.float32)
    sel[H:2 * H, 2 * H:3 * H] = msw
    return cosT, sinT, sel


def _build():
    import concourse.bacc as bacc
    import concourse.bass as bass
    import concourse.mybir as mybir
    from concourse import tile

    f32 = mybir.dt.float32
    f32r = mybir.dt.float32r
    bf16 = mybir.dt.bfloat16
    fp16 = mybir.dt.float16
    AF = mybir.ActivationFunctionType
    PSUM = bass.MemorySpace.PSUM

    nc = bacc.Bacc("TRN2", target_bir_lowering=False, debug=False,
                   num_devices=NCORES)

    # host-repacked so each load is 1 descriptor/partition: xq rows are
    # partitions, cols = 6 k-tiles x 512 positions, h=1 half then h=0 half
    xq_t = nc.dram_tensor("xq", [128, 2 * KT * 512], f32,
                          kind="ExternalInput")
    wb_t = nc.dram_tensor("wq", [128, KT * 2 * H], f32, kind="ExternalInput")
    b2_t = nc.dram_tensor("bias2", [2 * H, 1], f32, kind="ExternalInput")
    wo_t = nc.dram_tensor("w_o", [H, NPAIR, 2 * H], f32,
                          kind="ExternalInput")
    cos_t = nc.dram_tensor("cos_t", [H, S], f32, kind="ExternalInput")
    sin_t = nc.dram_tensor("sin_t", [H, S], f32, kind="ExternalInput")
    sel_t = nc.dram_tensor("sel3", [2 * H, 3 * H], f32, kind="ExternalInput")
    out_t = nc.dram_tensor("outF", [OH, 128, 4608], bf16,
                           kind="ExternalOutput")
    out_r = out_t.ap().rearrange("o p c -> p o c")

    def r(ap):
        return ap.bitcast(f32r)

    with tile.TileContext(nc) as tc:
        with tc.tile_pool(name="persist", bufs=1) as pp, \
             tc.tile_pool(name="scratch", bufs=2) as sp:
            wbT = pp.tile([128, KT, 2 * H], f32)
            sel3 = pp.tile([2 * H, 3 * H], f32)
            woP = pp.tile([H, NPAIR, 2 * H], f32)
            xTr1 = pp.tile([128, KT, 512], f32)
            xTr0 = pp.tile([128, KT, 512], f32)
            bias2 = pp.tile([2 * H, 1], f32)
            cosT = pp.tile([H, S], f32)
            sinT = pp.tile([H, S], f32)
            startR = pp.tile([H, S], f32)
            endAD = pp.tile([128, S], fp16)
            # fp16 stationary operand: halves LDWEIGHTS time via the PE's
            # fast-weight-load path (FWL needs a non-fp32 128-col weight);
            # fp16 keeps 11 mantissa bits so the K=64 cancellation in the
            # span contraction stays well under the error gate (bf16 here
            # measured 2e-2 per-elem, fp16 ~4e-3)
            tmpAD = [pp.tile([128, S], fp16, name=f"tmpAD{i}")
                     for i in range(2)]
            stg = pp.tile([128, OH, 4608], bf16)
            warmf = pp.tile([128, 512], f32)
            warm = pp.tile([128, 512], f32)
            wsink = pp.tile([1, 1], f32)

            nc.gpsimd.memset(warmf[:], 0.0)
            nc.vector.tensor_copy(r(warm[:]), warmf[:])
            # input DMAs split over both HWDGE rings, rotation tables
            # first so the rotation chain never waits on the bulk loads;
            # x halves split in two so projections start on the first three
            # k-tiles while the rest stream in
            nc.sync.dma_start(r(sel3[:]), r(sel_t.ap()))
            nc.sync.dma_start(sinT[:], sin_t.ap())
            nc.sync.dma_start(r(xTr1[:, 0:3, :]), r(xq_t.ap()[:, 0:1536]))
            nc.sync.dma_start(r(xTr1[:, 3:6, :]), r(xq_t.ap()[:, 1536:3072]))
            nc.scalar.dma_start(bias2[:], b2_t.ap())
            nc.scalar.dma_start(cosT[:], cos_t.ap())
            nc.scalar.dma_start(r(wbT[:]), r(wb_t.ap()))
            nc.scalar.dma_start(r(xTr0[:, 0:3, :]),
                                r(xq_t.ap()[:, 3072:4608]))
            nc.scalar.dma_start(r(xTr0[:, 3:6, :]),
                                r(xq_t.ap()[:, 4608:6144]))
            nc.scalar.dma_start(r(woP[:]), r(wo_t.ap()))

            with tc.tile_pool(name="psu", bufs=1, space=PSUM) as psu:
                # PE warm-up: a dense dummy-matmul burst during the input
                # loads trips the HAM activity monitor to 2.4 GHz before
                # the real stream starts; consumed so it can't be dropped.
                ps_w = psu.tile([128, 1024], f32, name="ps_w", tag="pair",
                                bufs=3)
                for i in range(NWARM):
                    nc.tensor.matmul(ps_w[:, 0:512], r(warm[:, 0:128]),
                                     r(warm[:]),
                                     start=(i == 0), stop=(i == NWARM - 1))
                nc.scalar.copy(wsink[:], ps_w[0:1, 0:1])

                def prep_h(h):
                    sl = slice(h * 512, (h + 1) * 512)
                    xTr = xTr1 if h == 1 else xTr0
                    ps2 = psu.tile([128, 512], f32, name="ps2", tag="small",
                                   bufs=2)
                    for kb in range(KT):
                        nc.tensor.matmul(
                            ps2[:], r(wbT[:, kb, :]), r(xTr[:, kb, :]),
                            start=(kb == 0), stop=(kb == KT - 1))
                    relu2 = sp.tile([128, 512], f32, name="relu2")
                    nc.scalar.activation(r(relu2[:]), ps2[:], AF.Relu,
                                         bias=bias2[:])
                    swS = psu.tile([H, 512], f32, name="swS", tag="small",
                                   bufs=2)
                    nc.tensor.matmul(swS[:], r(sel3[:, 0:H]), r(relu2[:]),
                                     start=True, stop=True)
                    exE = psu.tile([H, 512], f32, name="exE", tag="small",
                                   bufs=2)
                    nc.tensor.matmul(exE[:], r(sel3[:, H:2 * H]),
                                     r(relu2[:]), start=True, stop=True)
                    swE = psu.tile([H, 512], f32, name="swE", tag="small",
                                   bufs=2)
                    nc.tensor.matmul(swE[:], r(sel3[:, 2 * H:3 * H]),
                                     r(relu2[:]), start=True, stop=True)
                    # start rotation all on DVE (its tmp matmul gates the
                    # whole pair stream); the end side leans on GPSIMD
                    rm = sp.tile([H, 512], f32, name="rm")
                    nc.vector.tensor_mul(rm[:], relu2[0:H, :], cosT[:, sl])
                    rs = sp.tile([H, 512], f32, name="rs")
                    nc.vector.tensor_mul(rs[:], swS[:], sinT[:, sl])
                    nc.vector.tensor_add(r(startR[:, sl]), rm[:], rs[:])
                    rm2 = sp.tile([H, 512], f32, name="rm2")
                    nc.vector.tensor_mul(rm2[:], exE[:], cosT[:, sl])
                    rs2 = sp.tile([H, 512], f32, name="rs2")
                    nc.vector.tensor_mul(rs2[:], swE[:], sinT[:, sl])
                    nc.gpsimd.tensor_add(endAD[0:H, sl], rm2[:], rs2[:])
                    # duplicate endR onto partitions 64-127 for the
                    # odd-label (hi row group) span tiles
                    nc.scalar.dma_start(endAD[H:128, sl],
                                        endAD[0:H, sl])

                def cast(dst, src, eng="v"):
                    if eng == "v":
                        nc.vector.tensor_copy(dst, src)
                    else:
                        nc.scalar.copy(dst, src)

                def tmp_pair(j, h):
                    # one K=64 M=128 matmul: lhsT = [W_o(2j) | W_o(2j+1)]
                    # puts the even label's tmp on partitions 0-63 and the
                    # odd label's on 64-127 in one shot
                    sl = slice(h * 512, (h + 1) * 512)
                    ps_t = psu.tile([128, 512], f32, name="ps_t",
                                    tag="small", bufs=2)
                    nc.tensor.matmul(ps_t[:], r(woP[:, j, :]),
                                     r(startR[:, sl]), start=True, stop=True)
                    cast(tmpAD[j % 2][:, sl], ps_t[:], "v" if h else "a")

                def span_pair(j, xb, y0, y1, ps_e, ps_o, c0):
                    """span rows of block xb, cols [y0,y1), for labels
                    2j/2j+1 concurrently into psum cols [c0, c0+y1-y0)."""
                    t = tmpAD[j % 2]
                    w = y1 - y0
                    xc = slice(xb * 128, (xb + 1) * 128)
                    nc.tensor.matmul(ps_e[:, c0:c0 + w], t[0:H, xc],
                                     endAD[0:H, y0:y1], start=True,
                                     stop=True, tile_position=(0, 0))
                    nc.tensor.matmul(ps_o[:, c0:c0 + w], t[H:128, xc],
                                     endAD[H:128, y0:y1], start=True,
                                     stop=True, tile_position=(H, 0))

                def do_a(j):
                    # short row-blocks (xb>=4, h=1 data only): xb4+xb6 fill
                    # a 2-bank tile, xb5+xb7 pack into one bank
                    e1 = psu.tile([128, 1024], f32, name="e1", tag="pair",
                                  bufs=3)
                    o1 = psu.tile([128, 1024], f32, name="o1", tag="pair",
                                  bufs=3)
                    span_pair(j, 4, 512, 1024, e1, o1, 0)
                    span_pair(j, 6, 768, 1024, e1, o1, 512)
                    cast(stg[:, 2 * j, 0:768], e1[:, 0:768], "v")
                    cast(stg[:, 2 * j + 1, 0:768], o1[:, 0:768], "a")
                    e2 = psu.tile([128, 512], f32, name="e2", tag="small",
                                  bufs=2)
                    o2 = psu.tile([128, 512], f32, name="o2", tag="small",
                                  bufs=2)
                    span_pair(j, 5, 640, 1024, e2, o2, 0)
                    span_pair(j, 7, 896, 1024, e2, o2, 384)
                    cast(stg[:, 2 * j, 768:1280], e2[:], "v")
                    nc.sync.dma_start(out_r[:, 2 * j, 0:1280],
                                      stg[:, 2 * j, 0:1280])
                    cast(stg[:, 2 * j + 1, 768:1280], o2[:], "a")
                    nc.scalar.dma_start(out_r[:, 2 * j + 1, 0:1280],
                                        stg[:, 2 * j + 1, 0:1280])

                def do_b(j):
                    # long row-blocks: [h1 | h0] per 2-bank tile. Even
                    # label drains on the sync HWDGE ring, odd on the
                    # GPSIMD SWDGE queue, so the ACT engine never spends
                    # time dispatching output descriptors.
                    for xb in range(4):
                        w0 = 512 - 128 * xb
                        base = SEGS[4 + 2 * xb][3]
                        e = psu.tile([128, 1024], f32, name="eB",
                                     tag="pair", bufs=3)
                        o = psu.tile([128, 1024], f32, name="oB",
                                     tag="pair", bufs=3)
                        span_pair(j, xb, 512, 1024, e, o, 0)
                        span_pair(j, xb, 128 * xb, 512, e, o, 512)
                        cast(stg[:, 2 * j, base:base + 512 + w0],
                             e[:, 0:512 + w0], "v")
                        if xb == 3:
                            nc.sync.dma_start(out_r[:, 2 * j, 1280:4608],
                                              stg[:, 2 * j, 1280:4608])
                        cast(stg[:, 2 * j + 1, base:base + 512 + w0],
                             o[:, 0:512 + w0], "a")
                    nc.scalar.dma_start(out_r[:, 2 * j + 1, 1280:4608],
                                        stg[:, 2 * j + 1, 1280:4608])

                # interleaved so the PE never idles: h=0 prep and the next
                # pair's tmps slot between a pair's A and B chunk streams
                prep_h(1)
                tmp_pair(0, 1)
                prep_h(0)
                do_a(0)
                tmp_pair(1, 1)
                tmp_pair(0, 0)
                do_a(1)
                do_b(0)
                tmp_pair(2, 1)
                tmp_pair(1, 0)
                do_a(2)
                do_b(1)
                tmp_pair(3, 1)
                tmp_pair(2, 0)
                do_a(3)
                do_b(2)
                tmp_pair(3, 0)
                do_b(3)

    nc.compile()
    return nc


def _get_nc():
    if "nc" not in _STATE:
        _STATE["nc"] = _build()
    return _STATE["nc"]


def _make_in_maps(x, mask, W_start, b_start, W_end, b_end, weight):
    cosT, sinT, sel = _tables()
    x = np.asarray(x, np.float32)
    W_start = np.asarray(W_start, np.float32)
    W_end = np.asarray(W_end, np.float32)
    w_both = np.ascontiguousarray(np.concatenate([W_start, W_end], axis=1))
    # [128, KT*128]: row p holds W rows {t*128+p} back to back
    wq = np.ascontiguousarray(
        w_both.reshape(KT, 128, 2 * H).transpose(1, 0, 2).reshape(128, -1))
    bias2 = np.ascontiguousarray(
        np.concatenate([np.asarray(b_start, np.float32).reshape(H),
                        np.asarray(b_end, np.float32).reshape(H)]).reshape(
                            2 * H, 1))
    weight = np.ascontiguousarray(np.asarray(weight, np.float32))
    in_maps = []
    for c in range(NCORES):
        b, half = c // 2, c % 2
        # [128, 6144]: row p = 6 k-tiles of the h=1 half then of the h=0
        # half, so each projection half loads as one linear descriptor/row
        xp = x[b].T.reshape(KT, 128, S).transpose(1, 0, 2)  # [128, KT, S]
        xq = np.ascontiguousarray(np.concatenate(
            [xp[:, :, 512:].reshape(128, -1), xp[:, :, :512].reshape(128, -1)],
            axis=1))
        # [64, NPAIR, 128]: lhsT stacking the pair's two W_o side by side
        wg = weight[half * OH:(half + 1) * OH]  # [OH, H, H]
        woP = np.ascontiguousarray(
            wg.reshape(NPAIR, 2, H, H).transpose(2, 0, 1, 3).reshape(
                H, NPAIR, 2 * H))
        in_maps.append({
            "xq": xq,
            "wq": wq,
            "bias2": bias2,
            "w_o": woP,
            "cos_t": cosT,
            "sin_t": sinT,
            "sel3": sel,
        })
    return in_maps


def _execute(in_maps, trace=False):
    from concourse.bass_utils import run_bass_kernel_spmd
    nc = _get_nc()
    return run_bass_kernel_spmd(nc, in_maps, list(range(NCORES)), trace=trace)


_TRIL128 = np.tril(np.ones((128, 128), dtype=bool), k=-1)


def _assemble(core_outs, mask):
    """Unshard: scatter the device's packed upper blocks into the full
    [B, O, S, S] span tensor and materialize the mask/tril constants."""
    mask = np.asarray(mask, np.float32)
    full = np.empty((B, O, S, S), np.float32)
    # below-diagonal constant per column: -(1-pad)*NEG - NEG (exact in f32
    # because |span*pad| << 0.5*ulp(NEG))
    below = (mask.astype(np.float64) * NEG - 2.0 * NEG).astype(np.float32)
    for c in range(NCORES):
        b, half = c // 2, c % 2
        osl = slice(half * OH, (half + 1) * OH)
        outF = np.asarray(core_outs[c]["outF"])  # [OH, 128, 4608] bf16
        plain = bool(np.all(mask[b] == 1.0))
        for xb, y0, y1, col in SEGS:
            blk = outF[:, :, col:col + y1 - y0].astype(np.float32)
            if not plain:
                pad = mask[b, y0:y1][None, None, :]
                blk = blk * pad - (1.0 - pad) * np.float32(NEG)
            r0 = 128 * xb
            if y0 == r0:
                # this segment starts at the diagonal block: restore the
                # exact below-diagonal constants inside it
                blk[:, :, 0:128] = np.where(
                    _TRIL128[None], below[b, None, None, r0:r0 + 128],
                    blk[:, :, 0:128])
            full[b, osl, r0:r0 + 128, y0:y1] = blk
        for xb in range(1, 8):
            r0 = 128 * xb
            full[b, osl, r0:r0 + 128, 0:r0] = below[b, None, None, 0:r0]
    return full


def kernel(x, mask, W_start, b_start, W_end, b_end, weight):
    in_maps = _make_in_maps(x, mask, W_start, b_start, W_end, b_end, weight)
    res = _execute(in_maps)
    return _assemble([res.results[c] for c in range(NCORES)], mask)


# revision 30
# speedup vs baseline: 1.1004x; 1.0479x over previous
"""Biaffine span classifier kernel for 8 Trainium2 NeuronCores.

Math (per batch b, label o):
    start = relu(x @ W_start + b_start); end = relu(x @ W_end + b_end)
    rotate both with tiled-halves sinusoidal tables
    span[o,x,y] = startR[x,:] @ weight[o] @ endR[y,:]^T
    span = span*pad[y] - (1-pad[y])*NEG - NEG*tril(x>y)

Sharding: core c = b*2 + half handles batch b and labels [half*8, half*8+8).

The kernel is output-DMA / PSUM-drain bound, so the device writes only the
information-bearing part of each [S, S] span map: the 36 (of 64) 128x128
blocks on or above the diagonal, in bf16 (per-elem tolerance is 2e-2; bf16
adds ~2e-3), packed per label and shipped as one linear 1.18MB DMA per
label, alternating between the two HWDGE rings per label pair. Everything
below the diagonal is a mask-derived constant in fp32 (|span| <<
0.5*ulp(NEG), so the reference's `span - NEG` is exactly -NEG); the host
materializes those constants while unsharding, plus the below-diagonal
triangle inside the eight diagonal blocks and the mask terms for any
masked-out columns (the graded mask is all-ones, so that path is a no-op).

On-chip layout is transposed ([H, S], H on partitions). Matmuls run in
fp32r; operands are DMA-loaded straight into f32r-typed views. Labels are
processed in PAIRS: tmp = [W_o(2j) | W_o(2j+1)] @ startR is one K=64 M=128
matmul whose output rows 0-63/64-127 are the two labels' tmps, and the
K=64 biaffine span contractions of the pair run CONCURRENTLY in PE array
row groups (0,0)/(64,0) via tile_position, halving span matmul time. endR
is duplicated onto partitions 64-127 by an SBUF->SBUF DMA. PSUM->SBUF bf16
casts are merged across adjacent banks (chunk order [h1 | h0], and the two
short row-blocks packed into one bank) so each label needs only 6 casts;
DVE and ACT alternate casts, and the rotation's SBUF-only multiplies run
on GPSIMD. Inputs are host-repacked so every load is one descriptor per
partition (HWDGE dispatch cost scales with descriptor count). A ~3.4us
dense burst of dummy matmuls at kernel start trips the PE's HAM activity
monitor to 2.4 GHz while the inputs stream in.
"""

import numpy as np

B, S, I, H, O = 4, 1024, 768, 64, 16
NCORES = 8
OH = O // 2  # 8 labels per core
NPAIR = OH // 2
NEG = 1.0e12
KT = I // 128  # 6 k-tiles over the input dim
NWARM = 8

# packed staging segments: (xb, y0, y1, col). Row-block xb covers output
# rows [128*xb, 128*(xb+1)); the segment holds columns [y0, y1) at staging
# columns [col, col + y1 - y0). Long row-blocks are packed [h1 | h0] so a
# chunk pair casts as one contiguous op.
SEGS = [
    (4, 512, 1024, 0), (6, 768, 1024, 512), (5, 640, 1024, 768),
    (7, 896, 1024, 1152),
    (0, 512, 1024, 1280), (0, 0, 512, 1792),
    (1, 512, 1024, 2304), (1, 128, 512, 2816),
    (2, 512, 1024, 3200), (2, 256, 512, 3712),
    (3, 512, 1024, 3968), (3, 384, 512, 4480),
]

_STATE = {}


def _tables():
    """Host-precomputed constants (mimic reference fp32 ops)."""
    position = np.arange(S, dtype=np.float32)
    idx = np.arange(H // 2, dtype=np.float32)
    expo = (np.float32(-2.0) * idx) / np.float32(H)
    inv_freq = np.power(np.float32(10000.0), expo).astype(np.float32)
    ang = position[:, None] * inv_freq[None, :]          # [S, 32] f32
    cos_h = np.cos(ang).astype(np.float32).T             # [32, S]
    sin_h = np.sin(ang).astype(np.float32).T
    cosT = np.ascontiguousarray(np.concatenate([cos_h, cos_h], axis=0))  # [64, S]
    sinT = np.ascontiguousarray(np.concatenate([sin_h, sin_h], axis=0))
    # pair-swap as lhsT: out[2m] = -in[2m+1]; out[2m+1] = in[2m]
    msw = np.zeros((H, H), np.float32)
    for m in range(H // 2):
        msw[2 * m + 1, 2 * m] = -1.0
        msw[2 * m, 2 * m + 1] = 1.0
    # selectors on the stacked [start; end] projection (lhsT, [128, 192]):
    # [:, 0:64] swap start rows; [:, 64:128] extract end rows; [:, 128:192]
    # swap end rows
    sel = np.zeros((2 * H, 3 * H), np.float32)
    sel[0:H, 0:H] = msw
    sel[H:2 * H, H:2 * H] = np.eye(H, dtype=np.float32)
    sel[H:2 * H, 2 * H:3 * H] = msw
    return cosT, sinT, sel


def _build():
    import concourse.bacc as bacc
    import concourse.bass as bass
    import concourse.mybir as mybir
    from concourse import tile

    f32 = mybir.dt.float32
    f32r = mybir.dt.float32r
    bf16 = mybir.dt.bfloat16
    fp16 = mybir.dt.float16
    AF = mybir.ActivationFunctionType
    PSUM = bass.MemorySpace.PSUM

    nc = bacc.Bacc("TRN2", target_bir_lowering=False, debug=False,
                   num_devices=NCORES)

    # host-repacked so each load is 1 descriptor/partition: xq rows are
    # partitions, cols = 6 k-tiles x 512 positions, h=1 half then h=0 half
    xq_t = nc.dram_tensor("xq", [128, 2 * KT * 512], f32,
                          kind="ExternalInput")
    wb_t = nc.dram_tensor("wq", [128, KT * 2 * H], f32, kind="ExternalInput")
    b2_t = nc.dram_tensor("bias2", [2 * H, 1], f32, kind="ExternalInput")
    wo_t = nc.dram_tensor("w_o", [H, NPAIR, 2 * H], f32,
                          kind="ExternalInput")
    cos_t = nc.dram_tensor("cos_t", [H, S], f32, kind="ExternalInput")
    sin_t = nc.dram_tensor("sin_t", [H, S], f32, kind="ExternalInput")
    sel_t = nc.dram_tensor("sel3", [2 * H, 3 * H], f32, kind="ExternalInput")
    out_t = nc.dram_tensor("outF", [OH, 128, 4608], bf16,
                           kind="ExternalOutput")
    out_r = out_t.ap().rearrange("o p c -> p o c")

    def r(ap):
        return ap.bitcast(f32r)

    with tile.TileContext(nc) as tc:
        with tc.tile_pool(name="persist", bufs=1) as pp, \
             tc.tile_pool(name="scratch", bufs=2) as sp:
            wbT = pp.tile([128, KT, 2 * H], f32)
            sel3 = pp.tile([2 * H, 3 * H], f32)
            woP = pp.tile([H, NPAIR, 2 * H], f32)
            xTr1 = pp.tile([128, KT, 512], f32)
            xTr0 = pp.tile([128, KT, 512], f32)
            bias2 = pp.tile([2 * H, 1], f32)
            cosT = pp.tile([H, S], f32)
            sinT = pp.tile([H, S], f32)
            startR = pp.tile([H, S], f32)
            endAD = pp.tile([128, S], fp16)
            # fp16 stationary operand: halves LDWEIGHTS time via the PE's
            # fast-weight-load path (FWL needs a non-fp32 128-col weight);
            # fp16 keeps 11 mantissa bits so the K=64 cancellation in the
            # span contraction stays well under the error gate (bf16 here
            # measured 2e-2 per-elem, fp16 ~4e-3)
            tmpAD = [pp.tile([128, S], fp16, name=f"tmpAD{i}")
                     for i in range(2)]
            stg = pp.tile([128, OH, 4608], bf16)
            warmf = pp.tile([128, 512], f32)
            warm = pp.tile([128, 512], f32)
            wsink = pp.tile([1, 1], f32)

            nc.gpsimd.memset(warmf[:], 0.0)
            nc.vector.tensor_copy(r(warm[:]), warmf[:])
            # input DMAs split over both HWDGE rings, rotation tables
            # first so the rotation chain never waits on the bulk loads;
            # x halves split in two so projections start on the first three
            # k-tiles while the rest stream in
            nc.sync.dma_start(r(sel3[:]), r(sel_t.ap()))
            nc.sync.dma_start(sinT[:], sin_t.ap())
            nc.sync.dma_start(r(xTr1[:]), r(xq_t.ap()[:, 0:3072]))
            nc.scalar.dma_start(bias2[:], b2_t.ap())
            nc.scalar.dma_start(cosT[:], cos_t.ap())
            nc.scalar.dma_start(r(wbT[:]), r(wb_t.ap()))
            nc.scalar.dma_start(r(xTr0[:]), r(xq_t.ap()[:, 3072:6144]))
            nc.scalar.dma_start(r(woP[:]), r(wo_t.ap()))

            with tc.tile_pool(name="psu", bufs=1, space=PSUM) as psu:
                # PE warm-up: a dense dummy-matmul burst during the input
                # loads trips the HAM activity monitor to 2.4 GHz before
                # the real stream starts; consumed so it can't be dropped.
                ps_w = psu.tile([128, 1024], f32, name="ps_w", tag="pair",
                                bufs=3)
                for i in range(NWARM):
                    nc.tensor.matmul(ps_w[:, 0:512], r(warm[:, 0:128]),
                                     r(warm[:]),
                                     start=(i == 0), stop=(i == NWARM - 1))
                nc.scalar.copy(wsink[:], ps_w[0:1, 0:1])

                def prep_h(h):
                    sl = slice(h * 512, (h + 1) * 512)
                    xTr = xTr1 if h == 1 else xTr0
                    ps2 = psu.tile([128, 512], f32, name="ps2", tag="small",
                                   bufs=2)
                    for kb in range(KT):
                        nc.tensor.matmul(
                            ps2[:], r(wbT[:, kb, :]), r(xTr[:, kb, :]),
                            start=(kb == 0), stop=(kb == KT - 1))
                    relu2 = sp.tile([128, 512], f32, name="relu2")
                    nc.scalar.activation(r(relu2[:]), ps2[:], AF.Relu,
                                         bias=bias2[:])
                    swS = psu.tile([H, 512], f32, name="swS", tag="small",
                                   bufs=2)
                    nc.tensor.matmul(swS[:], r(sel3[:, 0:H]), r(relu2[:]),
                                     start=True, stop=True)
                    exE = psu.tile([H, 512], f32, name="exE", tag="small",
                                   bufs=2)
                    nc.tensor.matmul(exE[:], r(sel3[:, H:2 * H]),
                                     r(relu2[:]), start=True, stop=True)
                    swE = psu.tile([H, 512], f32, name="swE", tag="small",
                                   bufs=2)
                    nc.tensor.matmul(swE[:], r(sel3[:, 2 * H:3 * H]),
                                     r(relu2[:]), start=True, stop=True)
                    # start rotation: rm on GPSIMD (SBUF-only), PSUM-fed
                    # ops on DVE
                    rm = sp.tile([H, 512], f32, name="rm")
                    nc.gpsimd.tensor_mul(rm[:], relu2[0:H, :], cosT[:, sl])
                    rs = sp.tile([H, 512], f32, name="rs")
                    nc.vector.tensor_mul(rs[:], swS[:], sinT[:, sl])
                    nc.gpsimd.tensor_add(r(startR[:, sl]), rm[:], rs[:])
                    rm2 = sp.tile([H, 512], f32, name="rm2")
                    nc.vector.tensor_mul(rm2[:], exE[:], cosT[:, sl])
                    rs2 = sp.tile([H, 512], f32, name="rs2")
                    nc.vector.tensor_mul(rs2[:], swE[:], sinT[:, sl])
                    nc.vector.tensor_add(endAD[0:H, sl], rm2[:], rs2[:])
                    # duplicate endR onto partitions 64-127 for the
                    # odd-label (hi row group) span tiles
                    nc.scalar.dma_start(endAD[H:128, sl],
                                        endAD[0:H, sl])

                cast_n = [0]

                def cast(dst, src, eng=None):
                    if cast_n[0] % 2 == 0:
                        nc.vector.tensor_copy(dst, src)
                    else:
                        nc.scalar.copy(dst, src)
                    cast_n[0] += 1

                def tmp_pair(j, h):
                    # one K=64 M=128 matmul: lhsT = [W_o(2j) | W_o(2j+1)]
                    # puts the even label's tmp on partitions 0-63 and the
                    # odd label's on 64-127 in one shot
                    sl = slice(h * 512, (h + 1) * 512)
                    ps_t = psu.tile([128, 512], f32, name="ps_t",
                                    tag="small", bufs=2)
                    nc.tensor.matmul(ps_t[:], r(woP[:, j, :]),
                                     r(startR[:, sl]), start=True, stop=True)
                    cast(tmpAD[j % 2][:, sl], ps_t[:])

                def span_pair(j, xb, y0, y1, ps_e, ps_o, c0):
                    """span rows of block xb, cols [y0,y1), for labels
                    2j/2j+1 concurrently into psum cols [c0, c0+y1-y0)."""
                    t = tmpAD[j % 2]
                    w = y1 - y0
                    xc = slice(xb * 128, (xb + 1) * 128)
                    nc.tensor.matmul(ps_e[:, c0:c0 + w], t[0:H, xc],
                                     endAD[0:H, y0:y1], start=True,
                                     stop=True, tile_position=(0, 0))
                    nc.tensor.matmul(ps_o[:, c0:c0 + w], t[H:128, xc],
                                     endAD[H:128, y0:y1], start=True,
                                     stop=True, tile_position=(H, 0))

                def do_a(j):
                    # short row-blocks (xb>=4, h=1 data only): xb4+xb6 fill
                    # a 2-bank tile, xb5+xb7 pack into one bank
                    e1 = psu.tile([128, 1024], f32, name="e1", tag="pair",
                                  bufs=3)
                    o1 = psu.tile([128, 1024], f32, name="o1", tag="pair",
                                  bufs=3)
                    span_pair(j, 4, 512, 1024, e1, o1, 0)
                    span_pair(j, 6, 768, 1024, e1, o1, 512)
                    cast(stg[:, 2 * j, 0:768], e1[:, 0:768])
                    cast(stg[:, 2 * j + 1, 0:768], o1[:, 0:768])
                    e2 = psu.tile([128, 512], f32, name="e2", tag="small",
                                  bufs=2)
                    o2 = psu.tile([128, 512], f32, name="o2", tag="small",
                                  bufs=2)
                    span_pair(j, 5, 640, 1024, e2, o2, 0)
                    span_pair(j, 7, 896, 1024, e2, o2, 384)
                    cast(stg[:, 2 * j, 768:1280], e2[:])
                    cast(stg[:, 2 * j + 1, 768:1280], o2[:])

                def do_b(j):
                    # long row-blocks: [h1 | h0] per 2-bank tile
                    ring = nc.sync if j % 2 == 0 else nc.scalar
                    for xb in range(4):
                        w0 = 512 - 128 * xb
                        base = SEGS[4 + 2 * xb][3]
                        e = psu.tile([128, 1024], f32, name="eB",
                                     tag="pair", bufs=3)
                        o = psu.tile([128, 1024], f32, name="oB",
                                     tag="pair", bufs=3)
                        span_pair(j, xb, 512, 1024, e, o, 0)
                        span_pair(j, xb, 128 * xb, 512, e, o, 512)
                        cast(stg[:, 2 * j, base:base + 512 + w0],
                             e[:, 0:512 + w0])
                        if xb == 3:
                            ring.dma_start(out_r[:, 2 * j, :],
                                           stg[:, 2 * j, :])
                        cast(stg[:, 2 * j + 1, base:base + 512 + w0],
                             o[:, 0:512 + w0])
                    ring.dma_start(out_r[:, 2 * j + 1, :],
                                   stg[:, 2 * j + 1, :])

                # interleaved so the PE never idles: h=0 prep and the next
                # pair's tmps slot between a pair's A and B chunk streams
                prep_h(1)
                tmp_pair(0, 1)
                prep_h(0)
                do_a(0)
                tmp_pair(1, 1)
                tmp_pair(0, 0)
                do_a(1)
                do_b(0)
                tmp_pair(2, 1)
                tmp_pair(1, 0)
                do_a(2)
                do_b(1)
                tmp_pair(3, 1)
                tmp_pair(2, 0)
                do_a(3)
                do_b(2)
                tmp_pair(3, 0)
                do_b(3)

    nc.compile()
    return nc


def _get_nc():
    if "nc" not in _STATE:
        _STATE["nc"] = _build()
    return _STATE["nc"]


def _make_in_maps(x, mask, W_start, b_start, W_end, b_end, weight):
    cosT, sinT, sel = _tables()
    x = np.asarray(x, np.float32)
    W_start = np.asarray(W_start, np.float32)
    W_end = np.asarray(W_end, np.float32)
    w_both = np.ascontiguousarray(np.concatenate([W_start, W_end], axis=1))
    # [128, KT*128]: row p holds W rows {t*128+p} back to back
    wq = np.ascontiguousarray(
        w_both.reshape(KT, 128, 2 * H).transpose(1, 0, 2).reshape(128, -1))
    bias2 = np.ascontiguousarray(
        np.concatenate([np.asarray(b_start, np.float32).reshape(H),
                        np.asarray(b_end, np.float32).reshape(H)]).reshape(
                            2 * H, 1))
    weight = np.ascontiguousarray(np.asarray(weight, np.float32))
    in_maps = []
    for c in range(NCORES):
        b, half = c // 2, c % 2
        # [128, 6144]: row p = 6 k-tiles of the h=1 half then of the h=0
        # half, so each projection half loads as one linear descriptor/row
        xp = x[b].T.reshape(KT, 128, S).transpose(1, 0, 2)  # [128, KT, S]
        xq = np.ascontiguousarray(np.concatenate(
            [xp[:, :, 512:].reshape(128, -1), xp[:, :, :512].reshape(128, -1)],
            axis=1))
        # [64, NPAIR, 128]: lhsT stacking the pair's two W_o side by side
        wg = weight[half * OH:(half + 1) * OH]  # [OH, H, H]
        woP = np.ascontiguousarray(
            wg.reshape(NPAIR, 2, H, H).transpose(2, 0, 1, 3).reshape(
                H, NPAIR, 2 * H))
        in_maps.append({
            "xq": xq,
            "wq": wq,
            "bias2": bias2,
            "w_o": woP,
            "cos_t": cosT,
            "sin_t": sinT,
            "sel3": sel,
        })
    return in_maps


def _execute(in_maps, trace=False):
    from concourse.bass_utils import run_bass_kernel_spmd
    nc = _get_nc()
    return run_bass_kernel_spmd(nc, in_maps, list(range(NCORES)), trace=trace)


_TRIL128 = np.tril(np.ones((128, 128), dtype=bool), k=-1)


def _assemble(core_outs, mask):
    """Unshard: scatter the device's packed upper blocks into the full
    [B, O, S, S] span tensor and materialize the mask/tril constants."""
    mask = np.asarray(mask, np.float32)
    full = np.empty((B, O, S, S), np.float32)
    # below-diagonal constant per column: -(1-pad)*NEG - NEG (exact in f32
    # because |span*pad| << 0.5*ulp(NEG))
    below = (mask.astype(np.float64) * NEG - 2.0 * NEG).astype(np.float32)
    for c in range(NCORES):
        b, half = c // 2, c % 2
        osl = slice(half * OH, (half + 1) * OH)
        outF = np.asarray(core_outs[c]["outF"])  # [OH, 128, 4608] bf16
        plain = bool(np.all(mask[b] == 1.0))
        for xb, y0, y1, col in SEGS:
            blk = outF[:, :, col:col + y1 - y0].astype(np.float32)
            if not plain:
                pad = mask[b, y0:y1][None, None, :]
                blk = blk * pad - (1.0 - pad) * np.float32(NEG)
            r0 = 128 * xb
            if y0 == r0:
                # this segment starts at the diagonal block: restore the
                # exact below-diagonal constants inside it
                blk[:, :, 0:128] = np.where(
                    _TRIL128[None], below[b, None, None, r0:r0 + 128],
                    blk[:, :, 0:128])
            full[b, osl, r0:r0 + 128, y0:y1] = blk
        for xb in range(1, 8):
            r0 = 128 * xb
            full[b, osl, r0:r0 + 128, 0:r0] = below[b, None, None, 0:r0]
    return full


def kernel(x, mask, W_start, b_start, W_end, b_end, weight):
    in_maps = _make_in_maps(x, mask, W_start, b_start, W_end, b_end, weight)
    res = _execute(in_maps)
    return _assemble([res.results[c] for c in range(NCORES)], mask)
